# revision 1
# baseline (speedup 1.0000x reference)
"""MeshPotential (P3M) Trainium2 kernel, v3: spectral truncation + bf16 +
host-folded spread/gather.

The Coulomb kernel G(k) = 4*pi*exp(-sigma^2 k^2/2)/k^2 decays as
exp(-0.0079 n^2) on this mesh, so only modes |n| <= K=32 survive
(truncation rel err ~3e-6).  All DFTs are bf16 matmuls (1 cyc/row on PE)
with f32 PSUM accumulation.

Key folding (matmul associativity, done on host in f64):
  spread+z-DFT+y-DFT:  Y = F_y^T (S1l^T S1r) = (S1l F_y)^T S1r
  y-inv+z-inv+gather:  out = s6l^T (F_y^inv^T P) = (F_y^inv s6l)^T P
so the device runs only 4 matmul stages:
  P2' fused spread+y-DFT   -> Y(ky, x, kz)        (per-channel x-bin packs)
  P3  x-DFT                -> X(kx, ky, kz)
  G   multiply (fused into the PSUM->SBUF cast)
  P4  x-inverse            -> P(x, ky, kz)
  P6' fused y-inv+gather   -> per-atom partials   (all-atom x-bin packs)

Distribution (8 cores, SPMD, no collectives): core = (channel, kz-half),
kz halves of 0..32.  Host folds bins/halves into (n_atoms, n_species).
Intermediates y2/p4 live in DRAM as (ky, x, [re|im]) so crossing DMAs
have runs >= 68B.
"""

import numpy as np
import ml_dtypes

import concourse.bass as bass  # noqa: F401
import concourse.mybir as mybir
import concourse.tile as tile
from concourse import bacc
from concourse.bass_utils import run_bass_kernel_spmd

F32 = mybir.dt.float32
BF16 = mybir.dt.bfloat16
NPBF = ml_dtypes.bfloat16

NS = 256
K = 32
KK = 2 * K + 1          # 65 modes -K..K for kx, ky
KZF = K + 1             # 33 kz modes 0..K
KZC = 17                # kz slab per core (h=0: 0..16, h=1: 17..32 + pad)
KRI = 2 * KZC           # 34, [re|im] packed
XB = 16                 # x batch for y2 writes / p4 reads
TKY = 13                # ky tile in phases 3/4 (5 tiles cover 65)
NTK = 5
N_CORES = 8
SMEARING = 0.4

_cache = {}


def build_program(C1, P1p, C6, P6p):
    PC1, PC6 = P1p * C1, P6p * C6
    NPACK1 = NS // P1p
    NPACK6 = NS // P6p
    FR1 = P1p * KRI         # P2' psum free (re+im)
    FH1 = P1p * KZC         # per-matmul free (one of re/im)
    FR6 = P6p * KRI
    FH6 = P6p * KZC
    NCH = 4                  # fs1/s1r load chunks (compute starts after 1st)
    CPK = NPACK1 // NCH
    nc = bacc.Bacc(None, target_bir_lowering=False, debug=False)
    dp = lambda name, shape, dt=BF16: nc.declare_dram_parameter(
        name, list(shape), dt, isOutput=False)
    # folded spread lhsT packs: [FcS1 | FsS1 | FnS1]
    fs1 = dp("fs1", (PC1, NPACK1, 3 * KK))
    s1r = dp("s1r", (PC1, NPACK1, P1p, KRI))
    # folded gather lhsT packs: [ICS6 | ISS6 | INS6]
    fs6 = dp("fs6", (KK, NPACK6, 3 * PC6))
    s6r = dp("s6r", (PC6, NPACK6, P6p, KRI))
    fwdc = dp("fwdc", (2, 128, KK))
    fwds = dp("fwds", (2, 128, KK))
    fwdn = dp("fwdn", (2, 128, KK))
    invc = dp("invc", (KK, NS))
    invs = dp("invs", (KK, NS))
    invn = dp("invn", (KK, NS))
    gsp = dp("gs", (KK, KK, KRI))             # G duplicated on re|im halves
    outp = nc.declare_dram_parameter("out", [PC6, NPACK6], F32, isOutput=True)
    mult = mybir.AluOpType.mult
    add = mybir.AluOpType.add

    with tile.TileContext(nc) as tc:
        with (
            tc.tile_pool(name="constp", bufs=1) as constp,
            tc.tile_pool(name="iop", bufs=3) as iop,
            tc.tile_pool(name="psp", bufs=4, space="PSUM") as psp,
            tc.tile_pool(name="dramp", bufs=1, space="DRAM") as dramp,
        ):
            # P2'-critical packs load in NCH chunks on the sync queue so the
            # first matmuls start after ~1/NCH of the bytes; everything else
            # streams on the scalar queue during compute, in need-order.
            FS1C, S1RC = [], []
            for c in range(NCH):
                csl = slice(c * CPK, (c + 1) * CPK)
                t1 = constp.tile([PC1, CPK, 3 * KK], BF16, name=f"fs1c{c}")
                nc.sync.dma_start(t1[:], fs1[:, csl, :])
                FS1C.append(t1)
                t2 = constp.tile([PC1, CPK, P1p, KRI], BF16, name=f"s1rc{c}")
                nc.sync.dma_start(t2[:], s1r[:, csl, :, :])
                S1RC.append(t2)
            FC, FS, FN = [], [], []
            for ch in range(2):
                for nm, lst, par in (("fc", FC, fwdc), ("fs", FS, fwds),
                                     ("fn", FN, fwdn)):
                    t_ = constp.tile([128, KK], BF16, name=f"{nm}{ch}")
                    nc.scalar.dma_start(t_[:], par[ch])
                    lst.append(t_)
            IC = constp.tile([KK, NS], BF16)
            nc.scalar.dma_start(IC[:], invc[:])
            IS = constp.tile([KK, NS], BF16)
            nc.scalar.dma_start(IS[:], invs[:])
            IN = constp.tile([KK, NS], BF16)
            nc.scalar.dma_start(IN[:], invn[:])
            GS = constp.tile([KK, KK, KRI], BF16)
            FS6 = constp.tile([KK, NPACK6, 3 * PC6], BF16)
            S6R = constp.tile([PC6, NPACK6, P6p, KRI], BF16)
            OUT = constp.tile([PC6, NPACK6], F32)

            # y2 as 2D (ky*kri, x): written once from the SBUF accumulator,
            # read back via the DMA crossbar transpose.  Rows padded to a
            # multiple of 16 (xbar tile height); pad rows are never read.
            KKP = 72
            y2a = dramp.tile([KKP * KRI, 128], BF16)
            y2b = dramp.tile([KKP * KRI, 128], BF16)
            # p4 as 2D (x*kri, ky): same trick for the second crossing.
            # ky padded to 128 cols to satisfy the xbar min-width on readback.
            KYP = 128
            p4h = [dramp.tile([128 * KRI, KYP], BF16, name=f"p4{c}")
                   for c in range(2)]
            YTFa = constp.tile([KK, KRI, 128], BF16)
            YTFb = constp.tile([KK, KRI, 128], BF16)
            PTF = [constp.tile([128, KRI, KYP], BF16, name=f"ptf{c}")
                   for c in range(2)]

            # PSUM-capable engines only (GpSimd cannot touch PSUM)
            st = {"c": 0}

            def cop(dst, src):
                if st["c"] % 2 == 0:
                    nc.scalar.copy(dst, src)
                else:
                    nc.vector.tensor_copy(dst, src)
                st["c"] += 1

            mm = nc.tensor.matmul
            PPB1 = XB // P1p
            PPB6 = XB // P6p

            # ------- P2': fused spread + z-DFT + y-DFT, per pack ----------
            for pk in range(NPACK1):
                c, pc = divmod(pk, CPK)
                ps = psp.tile([KK, P1p, KRI], F32, tag="AB"[pk % 2])
                Zr = S1RC[c][:, pc, :, 0:KZC]
                Zi = S1RC[c][:, pc, :, KZC:KRI]
                mm(ps[:, :, 0:KZC], FS1C[c][:, pc, 0:KK], Zr, start=True, stop=False)
                mm(ps[:, :, 0:KZC], FS1C[c][:, pc, KK:2 * KK], Zi, start=False, stop=True)
                mm(ps[:, :, KZC:KRI], FS1C[c][:, pc, 0:KK], Zi, start=True, stop=False)
                mm(ps[:, :, KZC:KRI], FS1C[c][:, pc, 2 * KK:3 * KK], Zr, start=False, stop=True)
                # transposed copy into the (ky, kri, x) accumulator
                half, xoff = divmod(pk * P1p, 128)
                ytf = YTFa if half == 0 else YTFb
                cop(ytf[:, :, xoff:xoff + P1p], ps[:].transpose([0, 2, 1]))
                if pk == NPACK1 // 2 - 1:
                    nc.sync.dma_start(y2a[0:KK * KRI, :], YTFa[:])
                    nc.scalar.dma_start(GS[:], gsp[:])
            nc.sync.dma_start(y2b[0:KK * KRI, :], YTFb[:])

            # ------- P3: x-DFT, G ; P4: x-inverse -------------------------
            y3 = constp.tile([128, 2, KKP, KRI], BF16)
            nc.sync.dma_start_transpose(y3[:, 0], y2a[:])
            nc.sync.dma_start_transpose(y3[:, 1], y2b[:])
            nc.scalar.dma_start(FS6[:], fs6[:])
            xtf = constp.tile([KK, KK, KRI], BF16)
            for t in range(NTK):
                ky0 = TKY * t
                px = psp.tile([KK, TKY, KRI], F32, tag="A")
                ksl = slice(ky0, ky0 + TKY)
                mm(px[:, :, 0:KZC], FC[0][:], y3[:, 0, ksl, 0:KZC], start=True, stop=False)
                mm(px[:, :, 0:KZC], FC[1][:], y3[:, 1, ksl, 0:KZC], start=False, stop=False)
                mm(px[:, :, 0:KZC], FS[0][:], y3[:, 0, ksl, KZC:KRI], start=False, stop=False)
                mm(px[:, :, 0:KZC], FS[1][:], y3[:, 1, ksl, KZC:KRI], start=False, stop=True)
                mm(px[:, :, KZC:KRI], FC[0][:], y3[:, 0, ksl, KZC:KRI], start=True, stop=False)
                mm(px[:, :, KZC:KRI], FC[1][:], y3[:, 1, ksl, KZC:KRI], start=False, stop=False)
                mm(px[:, :, KZC:KRI], FN[0][:], y3[:, 0, ksl, 0:KZC], start=False, stop=False)
                mm(px[:, :, KZC:KRI], FN[1][:], y3[:, 1, ksl, 0:KZC], start=False, stop=True)
                nc.vector.tensor_tensor(xtf[:, ky0:ky0 + TKY, :], px[:],
                                        GS[:, ky0:ky0 + TKY, :], op=mult)
            for ch in range(2):
                xsl = slice(128 * ch, 128 * (ch + 1))
                ptf = PTF[ch]
                for t in range(NTK):
                    ky0 = TKY * t
                    xt = xtf[:, ky0:ky0 + TKY, :]
                    pp_ = psp.tile([128, TKY, KRI], F32, tag="B")
                    mm(pp_[:, :, 0:KZC], IC[:, xsl], xt[:, :, 0:KZC], start=True, stop=False)
                    mm(pp_[:, :, 0:KZC], IN[:, xsl], xt[:, :, KZC:KRI], start=False, stop=True)
                    mm(pp_[:, :, KZC:KRI], IS[:, xsl], xt[:, :, 0:KZC], start=True, stop=False)
                    mm(pp_[:, :, KZC:KRI], IC[:, xsl], xt[:, :, KZC:KRI], start=False, stop=True)
                    # transposed copy into the (x, kri, ky) accumulator
                    cop(ptf[:, :, ky0:ky0 + TKY], pp_[:].transpose([0, 2, 1]))
                dst = p4h[ch][:]
                nc.sync.dma_start(dst.rearrange("(x q) k -> x (q k)", q=KRI),
                                  ptf[:])
                if ch == 0:
                    nc.scalar.dma_start(S6R[:], s6r[:])

            # ------- P6': fused y-inverse + z-inverse + gather ------------
            XB6 = 32
            PPB6b = XB6 // P6p           # packs per batch
            for xi in range(NS // XB6):
                x0 = XB6 * xi
                wt = iop.tile([128, XB6, KRI], BF16, tag="wt")
                xh, xr = divmod(x0, 128)
                nc.sync.dma_start_transpose(
                    wt[:], p4h[xh][xr * KRI:(xr + XB6) * KRI, :])
                scr4 = iop.tile([PC6, PPB6b, P6p, KRI], BF16, tag="scr")
                for j in range(PPB6b // 2):   # two packs per PSUM tile
                    ps6 = psp.tile([PC6, 2, P6p, KRI], F32, tag="AB"[j % 2])
                    for u in range(2):
                        pp = 2 * j + u
                        pk = xi * PPB6b + pp
                        xsl = slice(pp * P6p, (pp + 1) * P6p)
                        Pr = wt[0:KK, xsl, 0:KZC]
                        Pi = wt[0:KK, xsl, KZC:KRI]
                        mm(ps6[:, u, :, 0:KZC], FS6[:, pk, 0:PC6], Pr,
                           start=True, stop=False)
                        mm(ps6[:, u, :, 0:KZC], FS6[:, pk, 2 * PC6:3 * PC6], Pi,
                           start=False, stop=True)
                        mm(ps6[:, u, :, KZC:KRI], FS6[:, pk, PC6:2 * PC6], Pr,
                           start=True, stop=False)
                        mm(ps6[:, u, :, KZC:KRI], FS6[:, pk, 0:PC6], Pi,
                           start=False, stop=True)
                    pk0 = xi * PPB6b + 2 * j
                    s6sl = S6R[:, pk0:pk0 + 2, :, :]
                    if j % 2 == 0:
                        nc.vector.tensor_tensor(scr4[:, 2 * j:2 * j + 2], ps6[:],
                                                s6sl, op=mult)
                    else:
                        # Act casts PSUM->bf16, gpsimd multiplies (SBUF-only)
                        ct = iop.tile([PC6, 2, P6p, KRI], BF16, tag="ct")
                        nc.scalar.copy(ct[:], ps6[:])
                        nc.gpsimd.tensor_tensor(scr4[:, 2 * j:2 * j + 2], ct[:],
                                                s6sl, op=mult)
                nc.vector.tensor_reduce(OUT[:, xi * PPB6b:(xi + 1) * PPB6b],
                                        scr4[:], axis=mybir.AxisListType.XY,
                                        op=add)
            nc.sync.dma_start(outp[:], OUT[:])
    nc.compile()
    return nc


def _weights_order4(x):
    x2 = x * x
    x3 = x2 * x
    return np.stack([
        (1 - 6 * x + 12 * x2 - 8 * x3) / 48,
        (23 - 30 * x - 12 * x2 + 24 * x3) / 48,
        (23 + 30 * x - 12 * x2 - 24 * x3) / 48,
        (1 + 6 * x + 12 * x2 + 8 * x3) / 48,
    ])


def _pack_pick(nbin):
    for p in (8, 4, 2, 1):
        if p * nbin <= 128:
            return p
    raise ValueError(f"bin count {nbin} > 128 unsupported")


def _blockdiag(dense, NPACK, P, C, W):
    """dense (NS, C, W) -> (P*C, NPACK, P*W) block-diagonal."""
    a5 = np.zeros((NPACK, P, C, P, W), dtype=dense.dtype)
    i = np.arange(P)
    a5[:, i, :, i, :] = dense.reshape(NPACK, P, C, W).transpose(1, 0, 2, 3)
    return a5.reshape(NPACK, P * C, P * W).transpose(1, 0, 2)


def host_prep(cell, positions, charges):
    NA = positions.shape[0]
    NSP = charges.shape[1]
    cell = np.asarray(cell, dtype=np.float64)
    positions = np.asarray(positions, dtype=np.float64)
    charges = np.asarray(charges, dtype=np.float64)

    inv_cell = np.linalg.inv(cell)
    pos_rel = NS * (positions @ inv_cell)
    idx0 = np.floor(pos_rel)
    t = pos_rel - (idx0 + 0.5)
    w = _weights_order4(t)                                   # (4, NA, 3)
    offs = np.arange(-1, 3)
    idx = (idx0.astype(np.int64)[None] + offs[:, None, None]) % NS

    kzf = np.arange(KZF)
    Sz = np.zeros((NA, KZF), dtype=np.complex128)
    for j in range(4):
        Sz += w[j, :, 2:3] * np.exp(-2j * np.pi * idx[j, :, 2:3] * kzf[None] / NS)
    Sy = np.zeros((NA, NS))
    for j in range(4):
        np.add.at(Sy, (np.arange(NA), idx[j, :, 1]), w[j, :, 1])

    slabs = [(0, KZC), (KZC, KZF)]
    wkz = np.where(kzf == 0, 1.0, 2.0)

    modes = np.arange(-K, K + 1)
    ab = (np.outer(np.arange(NS), modes)) % NS
    ang = 2 * np.pi * ab / NS
    cosm = np.cos(ang)                                       # (NS, KK) f64
    sinm = np.sin(ang)

    def zri_slab(zarr, lo, hi, weight):
        zr = zarr.real[:, lo:hi] * weight[lo:hi]
        zi = zarr.imag[:, lo:hi] * weight[lo:hi]
        pad = KZC - (hi - lo)
        return np.concatenate(
            [np.pad(zr, ((0, 0), (0, pad))), np.pad(zi, ((0, 0), (0, pad)))], axis=1)

    one = np.ones_like(wkz)

    # ---- gather bins: all atoms ----
    ex = idx[:, :, 0].ravel()
    en = np.tile(np.arange(NA), 4)
    ew = w[:, :, 0].ravel()
    order = np.argsort(ex, kind="stable")
    ex, en, ew = ex[order], en[order], ew[order]
    cnt6 = np.bincount(ex, minlength=NS)
    C6 = int(cnt6.max())
    slot6 = np.arange(len(ex)) - np.repeat(
        np.concatenate([[0], np.cumsum(cnt6)[:-1]]), cnt6)
    P6p = _pack_pick(C6)
    NPACK6 = NS // P6p
    PC6 = P6p * C6
    atom6 = np.full((NS, C6), -1, dtype=np.int64)
    atom6[ex, slot6] = en
    valid6 = atom6 >= 0

    s6l_d = np.zeros((NS, C6, NS))                           # (x, slot, y) f64
    s6l_d[ex, slot6] = ew[:, None] * Sy[en]
    # fold inverse-y DFT: ICS6[k, slot] = sum_y cos(y k) s6l[y, slot]
    ic6 = np.einsum("xsy,yk->xsk", s6l_d, cosm)              # (NS, C6, KK)
    is6 = np.einsum("xsy,yk->xsk", s6l_d, sinm)
    # pack to (KK, NPACK6, 3*PC6): [IC | IS | IN] blocks
    def pack6(m):                                            # (NS, C6, KK) ->
        return m.reshape(NPACK6, P6p * C6, KK).transpose(2, 0, 1)   # (KK, NPACK6, PC6)
    fs6 = np.concatenate([pack6(ic6), pack6(is6), pack6(-is6)],
                         axis=2).astype(NPBF)

    s6r_list = []
    for lo, hi in slabs:
        d = np.zeros((NS, C6, KRI), dtype=np.float64)
        d[ex, slot6] = zri_slab(Sz, lo, hi, wkz)[en]
        s6r_list.append(np.ascontiguousarray(
            _blockdiag(d.astype(np.float32), NPACK6, P6p, C6, KRI)
            .reshape(PC6, NPACK6, P6p, KRI)).astype(NPBF))

    # ---- spread bins: per channel ----
    C1 = 1
    chan_bins = []
    for c in range(NSP):
        q = charges[:, c]
        sel = np.where(q != 0)[0]
        cex = idx[:, sel, 0].ravel()
        cen = np.tile(sel, 4)
        cew = (w[:, sel, 0] * q[sel][None]).ravel()
        o = np.argsort(cex, kind="stable")
        cex, cen, cew = cex[o], cen[o], cew[o]
        cnt = np.bincount(cex, minlength=NS)
        C1 = max(C1, int(cnt.max()))
        cslot = np.arange(len(cex)) - np.repeat(
            np.concatenate([[0], np.cumsum(cnt)[:-1]]), cnt)
        chan_bins.append((cex, cen, cew, cslot))
    P1p = _pack_pick(C1)
    NPACK1 = NS // P1p
    PC1 = P1p * C1
    FR1 = P1p * KRI
    fs1_list = []
    s1r_list = []
    for c in range(NSP):
        cex, cen, cew, cslot = chan_bins[c]
        lhs_d = np.zeros((NS, C1, NS))
        lhs_d[cex, cslot] = cew[:, None] * Sy[cen]
        fc1 = np.einsum("xsy,yk->xsk", lhs_d, cosm)          # (NS, C1, KK)
        fs_1 = np.einsum("xsy,yk->xsk", lhs_d, sinm)
        def pack1(m):
            return m.reshape(NPACK1, P1p * C1, KK).transpose(1, 0, 2)  # (PC1, NPACK1, KK)
        fs1_list.append(np.concatenate(
            [pack1(fc1), pack1(fs_1), pack1(-fs_1)], axis=2).astype(NPBF))
        per_h = []
        for lo, hi in slabs:
            d = np.zeros((NS, C1, KRI), dtype=np.float64)
            d[cex, cslot] = zri_slab(Sz, lo, hi, one)[cen]
            per_h.append(np.ascontiguousarray(
                _blockdiag(d, NPACK1, P1p, C1, KRI)
                .reshape(PC1, NPACK1, P1p, KRI)).astype(NPBF))
        s1r_list.append(per_h)

    fwdc = np.ascontiguousarray(cosm.reshape(2, 128, KK)).astype(NPBF)
    fwds = np.ascontiguousarray(sinm.reshape(2, 128, KK)).astype(NPBF)
    fwdn = np.ascontiguousarray((-sinm).reshape(2, 128, KK)).astype(NPBF)
    invc = np.ascontiguousarray(cosm.T).astype(NPBF)
    invs = np.ascontiguousarray(sinm.T).astype(NPBF)
    invn = np.ascontiguousarray((-sinm.T)).astype(NPBF)

    recip = 2 * np.pi * inv_cell.T
    kx, ky, kz = np.meshgrid(modes.astype(np.float64), modes.astype(np.float64),
                             kzf.astype(np.float64), indexing="ij")
    kvec = kx[..., None] * recip[0] + ky[..., None] * recip[1] + kz[..., None] * recip[2]
    ksq = np.sum(kvec * kvec, axis=-1)
    G = np.where(ksq == 0, 0.0,
                 4 * np.pi * np.exp(-0.5 * SMEARING ** 2 * ksq)
                 / np.where(ksq == 0, 1.0, ksq))
    G = G / np.abs(np.linalg.det(cell))
    gs_list = []
    for lo, hi in slabs:
        g = np.pad(G[:, :, lo:hi], ((0, 0), (0, 0), (0, KZC - (hi - lo))))
        gs_list.append(np.ascontiguousarray(
            np.concatenate([g, g], axis=2)).astype(NPBF))   # dup re|im

    return dict(NA=NA, NSP=NSP, C1=C1, P1p=P1p, C6=C6, P6p=P6p,
                NPACK6=NPACK6, atom6=atom6, valid6=valid6,
                fs1=fs1_list, s1r=s1r_list, fs6=fs6, s6r=s6r_list,
                gs=gs_list, fwdc=fwdc, fwds=fwds, fwdn=fwdn,
                invc=invc, invs=invs, invn=invn)


def _run(cell, positions, charges, trace=False):
    prep = host_prep(cell, positions, charges)
    C1, P1p, C6, P6p = prep["C1"], prep["P1p"], prep["C6"], prep["P6p"]
    key = (C1, P1p, C6, P6p)
    if key not in _cache:
        _cache[key] = build_program(C1, P1p, C6, P6p)
    nc = _cache[key]

    in_maps = []
    for core in range(N_CORES):
        ch, h = divmod(core, 2)
        in_maps.append({
            "fs1": prep["fs1"][ch],
            "s1r": prep["s1r"][ch][h],
            "fs6": prep["fs6"],
            "s6r": prep["s6r"][h],
            "gs": prep["gs"][h],
            "fwdc": prep["fwdc"], "fwds": prep["fwds"], "fwdn": prep["fwdn"],
            "invc": prep["invc"], "invs": prep["invs"], "invn": prep["invn"],
        })
    res = run_bass_kernel_spmd(nc, in_maps, list(range(N_CORES)), trace=trace)

    NA, NSP = prep["NA"], prep["NSP"]
    atom6, valid6 = prep["atom6"], prep["valid6"]
    rows = (np.arange(NS) % P6p)[:, None] * C6 + np.arange(C6)[None]
    cols = np.broadcast_to((np.arange(NS) // P6p)[:, None], rows.shape)
    pot = np.zeros((NA, NSP), dtype=np.float64)
    for core in range(N_CORES):
        ch, h = divmod(core, 2)
        out = res.results[core]["out"]                      # (PC6, NPACK6)
        vals = out[rows, cols]
        np.add.at(pot[:, ch], atom6[valid6], vals[valid6])
    return pot.astype(np.float32), res


def kernel(cell, positions, charges):
    pot, _ = _run(cell, positions, charges, trace=False)
    return pot



# revision 9
# speedup vs baseline: 3.0615x; 3.0615x over previous
"""MeshPotential (P3M) Trainium2 kernel, v4: atom-direct truncated-mode
pipeline.

G(k) = 4*pi*exp(-sigma^2 k^2/2)/k^2 decays as exp(-0.0079 n^2) on this
mesh, so only modes |n| <= K survive.  With K small the whole
computation fits in mode space per atom -- no 256^3 mesh, no binning:

  rho_hat(k)  = sum_n q_n Sx[n,kx] Sy[n,ky] Sz[n,kz]     (spread)
  V(k)        = G(k) * rho_hat(k)                         (convolution)
  out_n       = sum_k Re( conj(S_n(k)) V(k) )             (gather)

with S the separable order-4 stencil DFT factors, computed on host.
Modes: kx,ky in -K..K (KK=2K+1), kz in 0..K with Hermitian weight 2.

Device stages per core (core = (channel, kz-half), 8 cores, SPMD):
  P1 spread   rhoT[u,kx] = sum_n T[n,u] SxW[n,kx]    u=(ky,kz), 4 mm/chunk
  P2 G-mult   Vt[u,{r,i,-i}] = G * rhoT              (DVE+gpsimd, psum cast)
  P3 gather   Wt[n,kx] = sum_u T6[u,n] Vt[u,kx]      atoms on out partitions
  P4 dot      out_n = sum_kx {SxR,-SxI}[n,kx] Wt[n,{r,i'},kx]   (DVE TTR)

All matmuls are bf16 with full 128-wide stationary operands; every
intermediate stays on chip.
"""

import numpy as np
import ml_dtypes

import concourse.bass as bass  # noqa: F401
import concourse.mybir as mybir
import concourse.tile as tile
from concourse import bacc
from concourse.bass_utils import run_bass_kernel_spmd

F32 = mybir.dt.float32
BF16 = mybir.dt.bfloat16
NPBF = ml_dtypes.bfloat16

NS = 256
SMEARING = 0.4
N_CORES = 8

K = 20
KK = 2 * K + 1            # 41 modes -K..K for kx, ky
KZ = K + 1                # 21 kz modes 0..K
KZC0 = (KZ + 1) // 2      # 11: h=0 slab 0..10
U = KK * KZC0             # 451, padded u size (h=1 uses 410 + zero pad)
UCH = [(i * 128, min(U, (i + 1) * 128)) for i in range((U + 127) // 128)]
NGC = 8                   # gather chunks of 128 atoms (1024 padded)

_cache = {}


def build_program(NSPC, use_ttr=False, use_gps=True):
    nc = bacc.Bacc(None, target_bir_lowering=False, debug=False)
    dp = lambda name, shape, dt=BF16: nc.declare_dram_parameter(
        name, list(shape), dt, isOutput=False)
    tsp = dp("tsp", (NSPC, 128, 2, U))          # spread lhsT: {Tr,Ti} per atom
    sxw = dp("sxw", (NSPC, 128, 3, KK))         # spread rhs: {SxR, SxI, -SxI}
    g3 = dp("g3", (U, 3, KK))                   # {G, G, -G} by u row
    t6 = dp("t6", (U, 2, 1024))                 # gather lhsT: {T6r,T6i} x atom
    sx6 = dp("sx6", (NGC, 128, 2, KK))          # dot in1: {SxR6, -SxI6}
    outp = nc.declare_dram_parameter("out", [128, NGC], F32, isOutput=True)
    mult = mybir.AluOpType.mult
    add = mybir.AluOpType.add

    with tile.TileContext(nc) as tc:
        with (
            tc.tile_pool(name="constp", bufs=1) as constp,
            tc.tile_pool(name="iop", bufs=2) as iop,
            tc.tile_pool(name="psp", bufs=1, space="PSUM") as psp,
        ):
            TSP, SXW = [], []
            for j in range(NSPC):
                t_ = constp.tile([128, 2, U], BF16, name=f"tsp{j}")
                nc.sync.dma_start(t_[:], tsp[j])
                TSP.append(t_)
                s_ = constp.tile([128, 3, KK], BF16, name=f"sxw{j}")
                nc.sync.dma_start(s_[:], sxw[j])
                SXW.append(s_)
            G3, T6 = [], []
            for ci, (u0, u1) in enumerate(UCH):
                us = u1 - u0
                g_ = constp.tile([128, 3, KK], BF16, name=f"g3{ci}")
                nc.scalar.dma_start(g_[0:us], g3[u0:u1])
                G3.append(g_)
            for ci, (u0, u1) in enumerate(UCH):
                us = u1 - u0
                t_ = constp.tile([128, 2, 1024], BF16, name=f"t6{ci}")
                nc.scalar.dma_start(t_[0:us], t6[u0:u1])
                T6.append(t_)
            SX6 = []
            for gi in range(NGC):
                s_ = constp.tile([128, 2, KK], BF16, name=f"sx6{gi}")
                nc.scalar.dma_start(s_[:], sx6[gi])
                SX6.append(s_)
            OUT = constp.tile([128, NGC], F32)
            VT = [constp.tile([128, 3, KK], BF16, name=f"vt{c}")
                  for c in range(len(UCH))]

            mm = nc.tensor.matmul

            # ---- P1 spread + P2 G-mult, per u-chunk --------------------
            ps_s = psp.tile([128, len(UCH), 2, KK], F32, tag="S")
            for ci, (u0, u1) in enumerate(UCH):
                us = u1 - u0
                usl = slice(u0, u1)
                # one accumulation group open per psum bank at a time
                for ri, (b0, b1) in enumerate(((0, 2), (1, 0))):
                    for j in range(NSPC):
                        mm(ps_s[0:us, ci, ri], TSP[j][:, 0, usl], SXW[j][:, b0],
                           start=j == 0, stop=False)
                        mm(ps_s[0:us, ci, ri], TSP[j][:, 1, usl], SXW[j][:, b1],
                           start=False, stop=j == NSPC - 1)
                # Vt_r = G*rho_r ; Vt_i = G*rho_i  (DVE, psum -> sbuf bf16)
                nc.vector.tensor_tensor(VT[ci][0:us, 0:2], ps_s[0:us, ci],
                                        G3[ci][0:us, 0:2], op=mult)
                # Vt_ni = -Vt_i (gpsimd, sbuf only)
                if use_gps:
                    nc.gpsimd.tensor_scalar_mul(VT[ci][0:us, 2],
                                                VT[ci][0:us, 1], -1.0)
                else:
                    nc.vector.tensor_tensor(VT[ci][0:us, 2:3],
                                            ps_s[0:us, ci, 1:2],
                                            G3[ci][0:us, 2:3], op=mult)

            # ---- P3 gather + P4 dot, per 128-atom chunk ----------------
            NUC = len(UCH)
            for gi in range(NGC):
                wt = psp.tile([128, 2, KK], F32, tag="AB"[gi % 2])
                gsl = slice(gi * 128, (gi + 1) * 128)
                for w, (l0, v0, l1, v1) in enumerate(((0, 0, 1, 1), (1, 0, 0, 2))):
                    for ci, (u0, u1) in enumerate(UCH):
                        us = u1 - u0
                        mm(wt[:, w], T6[ci][0:us, l0, gsl], VT[ci][0:us, v0],
                           start=ci == 0, stop=False)
                        mm(wt[:, w], T6[ci][0:us, l1, gsl], VT[ci][0:us, v1],
                           start=False, stop=ci == NUC - 1)
                scr = iop.tile([128, 2, KK], BF16, tag="AB"[gi % 2])
                if use_ttr:
                    nc.vector.tensor_tensor_reduce(
                        out=scr[:], in0=wt[:], in1=SX6[gi][:], scale=1.0,
                        scalar=0.0, op0=mult, op1=add,
                        accum_out=OUT[:, gi:gi + 1])
                else:
                    nc.vector.tensor_tensor(scr[:], wt[:], SX6[gi][:], op=mult)
                    nc.vector.tensor_reduce(OUT[:, gi:gi + 1], scr[:],
                                            axis=mybir.AxisListType.XY, op=add)
            nc.sync.dma_start(outp[:], OUT[:])
    nc.compile()
    return nc


def _weights_order4(x):
    x2 = x * x
    x3 = x2 * x
    return np.stack([
        (1 - 6 * x + 12 * x2 - 8 * x3) / 48,
        (23 - 30 * x - 12 * x2 + 24 * x3) / 48,
        (23 + 30 * x - 12 * x2 - 24 * x3) / 48,
        (1 + 6 * x + 12 * x2 + 8 * x3) / 48,
    ])


def host_prep(cell, positions, charges):
    NA = positions.shape[0]
    NSP = charges.shape[1]
    cell = np.asarray(cell, dtype=np.float64)
    positions = np.asarray(positions, dtype=np.float64)
    charges = np.asarray(charges, dtype=np.float64)

    inv_cell = np.linalg.inv(cell)
    pos_rel = NS * (positions @ inv_cell)
    idx0 = np.floor(pos_rel)
    t = pos_rel - (idx0 + 0.5)
    w = _weights_order4(t)                                   # (4, NA, 3)
    offs = np.arange(-1, 3)
    idx = (idx0.astype(np.int64)[None] + offs[:, None, None]) % NS

    mo = np.arange(-K, K + 1)
    moz = np.arange(0, K + 1)

    def dft(ax, modes):
        ph = np.exp(-2j * np.pi * idx[:, :, ax][..., None] * modes / NS)
        return np.einsum("jn,jnm->nm", w[:, :, ax], ph)      # (NA, M)

    Sx = dft(0, mo)
    Sy = dft(1, mo)
    Sz = dft(2, moz)

    # smeared Coulomb kernel on the truncated grid, Hermitian-z weight
    # and 1/det folded in
    recip = 2 * np.pi * inv_cell.T
    kxg, kyg, kzg = np.meshgrid(mo.astype(np.float64), mo.astype(np.float64),
                                moz.astype(np.float64), indexing="ij")
    kvec = (kxg[..., None] * recip[0] + kyg[..., None] * recip[1]
            + kzg[..., None] * recip[2])
    ksq = np.sum(kvec * kvec, axis=-1)
    G = np.where(ksq == 0, 0.0,
                 4 * np.pi * np.exp(-0.5 * SMEARING ** 2 * ksq)
                 / np.where(ksq == 0, 1.0, ksq))
    G = G / np.abs(np.linalg.det(cell))
    wkz = np.where(moz == 0, 1.0, 2.0)
    Gw = G * wkz                                             # (KK, KK, KZ)

    slabs = [(0, KZC0), (KZC0, KZ)]

    # gather-side atom blocks (all atoms, padded to 1024)
    sx6 = np.zeros((NGC, 128, 2, KK), dtype=NPBF)
    sx6[:, :, 0] = np.pad(Sx.real, ((0, 1024 - NA), (0, 0))).reshape(NGC, 128, KK)
    sx6[:, :, 1] = np.pad(-Sx.imag, ((0, 1024 - NA), (0, 0))).reshape(NGC, 128, KK)

    per_h = []
    for lo, hi in slabs:
        kzc = hi - lo
        TYZ = (Sy[:, :, None] * Sz[:, None, lo:hi]).reshape(NA, KK * kzc)
        t6 = np.zeros((U, 2, 1024), dtype=NPBF)
        t6[:KK * kzc, 0, :NA] = TYZ.real.T
        t6[:KK * kzc, 1, :NA] = TYZ.imag.T
        g3 = np.zeros((U, 3, KK), dtype=NPBF)
        gs = Gw[:, :, lo:hi].reshape(KK, KK * kzc).T         # (u, kx)
        g3[:KK * kzc, 0] = gs
        g3[:KK * kzc, 1] = gs
        g3[:KK * kzc, 2] = -gs
        per_h.append((TYZ, t6, g3))

    NSPC = 1
    sels = []
    for c in range(NSP):
        sel = np.where(charges[:, c] != 0)[0]
        sels.append(sel)
        NSPC = max(NSPC, (len(sel) + 127) // 128)

    per_core = []
    for c in range(NSP):
        sel = sels[c]
        q = charges[sel, c]
        SxW = Sx[sel] * q[:, None]
        sxw = np.zeros((NSPC, 128, 3, KK), dtype=NPBF)
        npad = NSPC * 128 - len(sel)
        sxw[:, :, 0] = np.pad(SxW.real, ((0, npad), (0, 0))).reshape(NSPC, 128, KK)
        sxw[:, :, 1] = np.pad(SxW.imag, ((0, npad), (0, 0))).reshape(NSPC, 128, KK)
        sxw[:, :, 2] = -sxw[:, :, 1]
        for h, (lo, hi) in enumerate(slabs):
            TYZ = per_h[h][0]
            tspa = np.zeros((NSPC, 128, 2, U), dtype=NPBF)
            kzc = hi - lo
            tspa[:, :, 0, :KK * kzc] = np.pad(
                TYZ.real[sel], ((0, npad), (0, 0))).reshape(NSPC, 128, KK * kzc)
            tspa[:, :, 1, :KK * kzc] = np.pad(
                TYZ.imag[sel], ((0, npad), (0, 0))).reshape(NSPC, 128, KK * kzc)
            per_core.append({
                "tsp": tspa, "sxw": sxw,
                "g3": per_h[h][2], "t6": per_h[h][1], "sx6": sx6,
            })
    return NSPC, NA, NSP, per_core


def _run(cell, positions, charges, trace=False):
    NSPC, NA, NSP, in_maps = host_prep(cell, positions, charges)
    if NSPC not in _cache:
        _cache[NSPC] = build_program(NSPC)
    nc = _cache[NSPC]

    res = run_bass_kernel_spmd(nc, in_maps, list(range(N_CORES)), trace=trace)

    pot = np.zeros((NA, NSP), dtype=np.float64)
    for core in range(N_CORES):
        c = core // 2
        out = res.results[core]["out"].astype(np.float64)    # (128, NGC)
        pot[:, c] += out.T.reshape(-1)[:NA]
    return pot.astype(np.float32), res


def kernel(cell, positions, charges):
    pot, _ = _run(cell, positions, charges, trace=False)
    return pot


# revision 11
# speedup vs baseline: 4.2680x; 1.3941x over previous
"""MeshPotential (P3M) Trainium2 kernel, v5: atom-direct truncated-mode
pipeline.

G(k) = 4*pi*exp(-sigma^2 k^2/2)/k^2 decays as exp(-0.0079 n^2) on this
mesh, so only modes |n| <= K survive.  With K small the whole
computation fits in mode space per atom -- no 256^3 mesh, no binning:

  rho_hat(k)  = sum_n q_n Sx[n,kx] Sy[n,ky] Sz[n,kz]     (spread)
  V(k)        = G(k) * rho_hat(k)                         (convolution)
  out_n       = sum_k Re( conj(S_n(k)) V(k) )             (gather)

with S the separable order-4 stencil DFT factors, computed on host.
Modes: kx,ky in -K..K (KK=2K+1), kz in 0..K with Hermitian weight 2.

Device stages per core (core = (channel, kz-half), 8 cores, SPMD):
  P1 spread   rhoT[u,kx] = sum_n T[n,u] SxW[n,kx]     u=(ky,kz) rows
  P2 G-mult   VTA[u] = [G*rho_r | -G*rho_i]           (DVE psum cast)
              VTB[u] = [G*rho_i |  G*rho_r]           (DVE + gpsimd copy)
  P3 gather   wt[n, {r,i'}] accumulates, per u-chunk ci:
                 + T6r[u,n]^T [Vr | Vni]    (one mm, 2*KK-wide rhs)
                 + T6i[u,n]^T [Vi | Vr ]    (one mm)
              giving  wt = [Re conj(T6)V | -Im conj(T6)V]  per atom
  P4 dot      out_n = sum_kx {SxR6,-SxI6} * wt        (DVE mult+reduce)

All matmuls run with full 128-row contraction and 128-wide stationary
operands; 8 consolidated input DMAs across 4 engine queues; every
intermediate stays on chip.
"""

import numpy as np
import ml_dtypes

import concourse.bass as bass  # noqa: F401
import concourse.mybir as mybir
import concourse.tile as tile
from concourse import bacc
from concourse.bass_utils import run_bass_kernel_spmd

F32 = mybir.dt.float32
BF16 = mybir.dt.bfloat16
NPBF = ml_dtypes.bfloat16

NS = 256
SMEARING = 0.4
N_CORES = 8

K = 18
KK = 2 * K + 1            # 37 modes -K..K for kx, ky
KZ = K + 1                # 19 kz modes 0..K
KZC0 = (KZ + 1) // 2      # 10: h=0 slab kz 0..9, h=1 slab 10..18
NUC = (KK * KZC0 + 127) // 128   # 3 u-chunks
U4 = NUC * 128            # 384, zero-padded u size
NGC = 8                   # gather chunks of 128 atoms (1024 padded)

_cache = {}


def build_program(NSPC):
    nc = bacc.Bacc(None, target_bir_lowering=False, debug=False)
    dp = lambda name, shape, dt=BF16: nc.declare_dram_parameter(
        name, list(shape), dt, isOutput=False)
    tsp = dp("tsp", (128, NSPC, 2, U4))         # spread lhsT: {Tr,Ti} per atom
    sxw = dp("sxw", (128, NSPC, 3, KK))         # spread rhs: {SxR, SxI, -SxI}
    g3 = dp("g3", (128, NUC, 3, KK))            # {G, -G, G} by u row
    t6 = dp("t6", (128, NUC, 2, 1024))          # gather lhsT: {T6r,T6i} x atom
    sx6 = dp("sx6", (128, NGC, 2, KK))          # dot in1: {SxR6, -SxI6}
    outp = nc.declare_dram_parameter("out", [128, NGC], F32, isOutput=True)
    mult = mybir.AluOpType.mult
    add = mybir.AluOpType.add

    with tile.TileContext(nc) as tc:
        with (
            tc.tile_pool(name="constp", bufs=1) as constp,
            tc.tile_pool(name="iop", bufs=2) as iop,
            tc.tile_pool(name="psp", bufs=1, space="PSUM") as psp,
        ):
            TSP = constp.tile([128, NSPC, 2, U4], BF16)
            SXW = constp.tile([128, NSPC, 3, KK], BF16)
            G3 = constp.tile([128, NUC, 3, KK], BF16)
            T6 = constp.tile([128, NUC, 2, 1024], BF16)
            SX6 = constp.tile([128, NGC, 2, KK], BF16)
            OUT = constp.tile([128, NGC], F32)
            VTA = constp.tile([128, NUC, 2, KK], BF16)
            VTB = constp.tile([128, NUC, 2, KK], BF16)

            # ---- consolidated input DMAs over 4 queues -----------------
            nc.sync.dma_start(TSP[:], tsp[:])
            nc.sync.dma_start(SXW[:], sxw[:])
            nc.scalar.dma_start(T6[:, 0], t6[:, 0])
            nc.gpsimd.dma_start(G3[:], g3[:])
            nc.gpsimd.dma_start(T6[:, 1], t6[:, 1])
            nc.scalar.dma_start(T6[:, 2], t6[:, 2])
            nc.scalar.dma_start(SX6[:], sx6[:])

            mm = nc.tensor.matmul

            # ---- P1 spread + P2 G-mult, per u-chunk --------------------
            ps_s = psp.tile([128, NUC, 2, KK], F32, tag="S")
            for ci in range(NUC):
                usl = slice(ci * 128, (ci + 1) * 128)
                # one accumulation group open per psum bank at a time
                for ri, (b0, b1) in enumerate(((0, 2), (1, 0))):
                    for j in range(NSPC):
                        mm(ps_s[:, ci, ri], TSP[:, j, 0, usl], SXW[:, j, b0],
                           start=j == 0, stop=False)
                        mm(ps_s[:, ci, ri], TSP[:, j, 1, usl], SXW[:, j, b1],
                           start=False, stop=j == NSPC - 1)
                # VTA = [G*rho_r | -G*rho_i]   (one DVE op, psum -> bf16)
                nc.vector.tensor_tensor(VTA[:, ci], ps_s[:, ci],
                                        G3[:, ci, 0:2], op=mult)
                # VTB = [G*rho_i | G*rho_r]
                nc.vector.tensor_tensor(VTB[:, ci, 0:1], ps_s[:, ci, 1:2],
                                        G3[:, ci, 2:3], op=mult)
                nc.gpsimd.tensor_scalar_mul(VTB[:, ci, 1], VTA[:, ci, 0], 1.0)

            # ---- P3 gather + P4 dot, per 128-atom chunk ----------------
            for gi in range(NGC):
                wt = psp.tile([128, 2, KK], F32, tag="AB"[gi % 2])
                gsl = slice(gi * 128, (gi + 1) * 128)
                for ci in range(NUC):
                    mm(wt[:], T6[:, ci, 0, gsl], VTA[:, ci],
                       start=ci == 0, stop=False)
                    mm(wt[:], T6[:, ci, 1, gsl], VTB[:, ci],
                       start=False, stop=ci == NUC - 1)
                scr = iop.tile([128, 2, KK], BF16, tag="AB"[gi % 2])
                nc.vector.tensor_tensor(scr[:], wt[:], SX6[:, gi], op=mult)
                nc.vector.tensor_reduce(OUT[:, gi:gi + 1], scr[:],
                                        axis=mybir.AxisListType.XY, op=add)
            nc.sync.dma_start(outp[:], OUT[:])
    nc.compile()
    return nc


def _weights_order4(x):
    x2 = x * x
    x3 = x2 * x
    return np.stack([
        (1 - 6 * x + 12 * x2 - 8 * x3) / 48,
        (23 - 30 * x - 12 * x2 + 24 * x3) / 48,
        (23 + 30 * x - 12 * x2 - 24 * x3) / 48,
        (1 + 6 * x + 12 * x2 + 8 * x3) / 48,
    ])


def host_prep(cell, positions, charges):
    NA = positions.shape[0]
    NSP = charges.shape[1]
    cell = np.asarray(cell, dtype=np.float64)
    positions = np.asarray(positions, dtype=np.float64)
    charges = np.asarray(charges, dtype=np.float64)

    inv_cell = np.linalg.inv(cell)
    pos_rel = NS * (positions @ inv_cell)
    idx0 = np.floor(pos_rel)
    t = pos_rel - (idx0 + 0.5)
    w = _weights_order4(t)                                   # (4, NA, 3)
    offs = np.arange(-1, 3)
    idx = (idx0.astype(np.int64)[None] + offs[:, None, None]) % NS

    mo = np.arange(-K, K + 1)
    moz = np.arange(0, K + 1)

    def dft(ax, modes):
        ph = np.exp(-2j * np.pi * idx[:, :, ax][..., None] * modes / NS)
        return np.einsum("jn,jnm->nm", w[:, :, ax], ph)      # (NA, M)

    Sx = dft(0, mo)
    Sy = dft(1, mo)
    Sz = dft(2, moz)

    # smeared Coulomb kernel on the truncated grid; Hermitian-z weight
    # and 1/det folded in
    recip = 2 * np.pi * inv_cell.T
    kxg, kyg, kzg = np.meshgrid(mo.astype(np.float64), mo.astype(np.float64),
                                moz.astype(np.float64), indexing="ij")
    kvec = (kxg[..., None] * recip[0] + kyg[..., None] * recip[1]
            + kzg[..., None] * recip[2])
    ksq = np.sum(kvec * kvec, axis=-1)
    G = np.where(ksq == 0, 0.0,
                 4 * np.pi * np.exp(-0.5 * SMEARING ** 2 * ksq)
                 / np.where(ksq == 0, 1.0, ksq))
    G = G / np.abs(np.linalg.det(cell))
    wkz = np.where(moz == 0, 1.0, 2.0)
    Gw = G * wkz                                             # (KK, KK, KZ)

    slabs = [(0, KZC0), (KZC0, KZ)]

    sx6 = np.zeros((1024, 2, KK), dtype=NPBF)
    sx6[:NA, 0] = Sx.real
    sx6[:NA, 1] = -Sx.imag
    sx6 = np.ascontiguousarray(
        sx6.reshape(NGC, 128, 2, KK).transpose(1, 0, 2, 3))  # (128,NGC,2,KK)

    per_h = []
    for lo, hi in slabs:
        kzc = hi - lo
        uu = KK * kzc
        TYZ = (Sy[:, :, None] * Sz[:, None, lo:hi]).reshape(NA, uu)
        t6 = np.zeros((U4, 2, 1024), dtype=NPBF)
        t6[:uu, 0, :NA] = TYZ.real.T
        t6[:uu, 1, :NA] = TYZ.imag.T
        t6 = np.ascontiguousarray(
            t6.reshape(NUC, 128, 2, 1024).transpose(1, 0, 2, 3))
        g3 = np.zeros((U4, 3, KK), dtype=NPBF)
        gs = Gw[:, :, lo:hi].reshape(KK, uu).T               # (u, kx)
        g3[:uu, 0] = gs
        g3[:uu, 1] = -gs
        g3[:uu, 2] = gs
        g3 = np.ascontiguousarray(
            g3.reshape(NUC, 128, 3, KK).transpose(1, 0, 2, 3))
        per_h.append((TYZ, t6, g3))

    NSPC = 1
    sels = []
    for c in range(NSP):
        sel = np.where(charges[:, c] != 0)[0]
        sels.append(sel)
        NSPC = max(NSPC, (len(sel) + 127) // 128)

    per_core = []
    for c in range(NSP):
        sel = sels[c]
        npad = NSPC * 128 - len(sel)
        q = charges[sel, c]
        SxW = Sx[sel] * q[:, None]
        sxw = np.zeros((NSPC * 128, 3, KK), dtype=NPBF)
        sxw[:len(sel), 0] = SxW.real
        sxw[:len(sel), 1] = SxW.imag
        sxw[:len(sel), 2] = -SxW.imag
        sxw = np.ascontiguousarray(
            sxw.reshape(NSPC, 128, 3, KK).transpose(1, 0, 2, 3))
        for h, (lo, hi) in enumerate(slabs):
            TYZ = per_h[h][0]
            uu = TYZ.shape[1]
            tspa = np.zeros((NSPC * 128, 2, U4), dtype=NPBF)
            tspa[:len(sel), 0, :uu] = TYZ.real[sel]
            tspa[:len(sel), 1, :uu] = TYZ.imag[sel]
            tspa = np.ascontiguousarray(
                tspa.reshape(NSPC, 128, 2, U4).transpose(1, 0, 2, 3))
            per_core.append({
                "tsp": tspa, "sxw": sxw,
                "g3": per_h[h][2], "t6": per_h[h][1], "sx6": sx6,
            })
    return NSPC, NA, NSP, per_core


def _run(cell, positions, charges, trace=False):
    NSPC, NA, NSP, in_maps = host_prep(cell, positions, charges)
    if NSPC not in _cache:
        _cache[NSPC] = build_program(NSPC)
    nc = _cache[NSPC]

    res = run_bass_kernel_spmd(nc, in_maps, list(range(N_CORES)), trace=trace)

    pot = np.zeros((NA, NSP), dtype=np.float64)
    for core in range(N_CORES):
        c = core // 2
        out = res.results[core]["out"].astype(np.float64)    # (128, NGC)
        pot[:, c] += out.T.reshape(-1)[:NA]
    return pot.astype(np.float32), res


def kernel(cell, positions, charges):
    pot, _ = _run(cell, positions, charges, trace=False)
    return pot


# revision 13
# speedup vs baseline: 5.1968x; 1.2176x over previous
"""MeshPotential (P3M) Trainium2 kernel, v6: atom-direct truncated-mode
pipeline with budgeted disk truncation.

G(k) = 4*pi*exp(-sigma^2 k^2/2)/k^2 decays as exp(-0.0079 n^2) on this
mesh, so only low modes survive.  The whole computation runs in mode
space per atom -- no 256^3 mesh, no binning:

  rho_hat(k)  = sum_n q_n Sx[n,kx] Sy[n,ky] Sz[n,kz]     (spread)
  V(k)        = G(k) * rho_hat(k)                         (convolution)
  out_n       = sum_k Re( conj(S_n(k)) V(k) )             (gather)

with S the separable order-4 stencil DFT factors, computed on host.
kx runs -K..K (KK wide); the (ky,kz) plane (kz>=0, Hermitian weight 2)
is truncated to the UTOT lowest-|ky,kz| rows and dealt evenly across
the two kz-half cores, 256 rows (2 psum chunks) each.

Device stages per core (core = (channel, row-half), 8 cores, SPMD):
  P1 spread   rhoT[u,kx] = sum_n T[n,u] SxW[n,kx]
  P2 G-mult   VTA[u] = [G*rho_r | -G*rho_i]      (DVE psum cast)
              VTB[u] = [G*rho_i |  G*rho_r]      (DVE + Act copy)
  P3 gather   wt[n, {r,i'}] += T6r[u,n]^T [Vr|Vni] + T6i[u,n]^T [Vi|Vr]
  P4 dot      out_n = sum_kx {SxR6,-SxI6} * wt   (DVE mult+reduce)

All matmuls use full 128-row contraction and 128-wide stationary
operands; 5 consolidated input DMAs over 3 queues; gather runs in two
4-bank waves interleaved per u-chunk so it overlaps the t6 stream.
"""

import numpy as np
import ml_dtypes

import concourse.bass as bass  # noqa: F401
import concourse.mybir as mybir
import concourse.tile as tile
from concourse import bacc
from concourse.bass_utils import run_bass_kernel_spmd

F32 = mybir.dt.float32
BF16 = mybir.dt.bfloat16
NPBF = ml_dtypes.bfloat16

NS = 256
SMEARING = 0.4
N_CORES = 8

K = 20
KK = 2 * K + 1            # 41 kx modes -K..K
KZ = K + 1                # kz 0..K (Hermitian weight 2 for kz>0)
UTOT = 512                # kept (ky,kz) rows, lowest ky^2+kz^2
U4 = UTOT // 2            # 256 rows per core
NUC = U4 // 128           # 2 u-chunks
NGC = 8                   # gather chunks of 128 atoms (1024 padded)
SPW = 2 * U4 + 3 * KK     # spr row: [Tr | Ti | SxR | SxI | -SxI]

_cache = {}


def build_program(NSPC):
    nc = bacc.Bacc(None, target_bir_lowering=False, debug=False)
    dp = lambda name, shape, dt=BF16: nc.declare_dram_parameter(
        name, list(shape), dt, isOutput=False)
    spr = dp("spr", (128, NSPC, SPW))           # spread lhsT+rhs, one DMA
    g3 = dp("g3", (128, NUC, 3, KK))            # {G, -G, G} by u row
    t6 = dp("t6", (128, NUC, 2, 1024))          # gather lhsT: {T6r,T6i} x atom
    sx6 = dp("sx6", (128, NGC, 2, KK))          # dot in1: {SxR6, -SxI6}
    outp = nc.declare_dram_parameter("out", [128, NGC], F32, isOutput=True)
    mult = mybir.AluOpType.mult
    add = mybir.AluOpType.add

    with tile.TileContext(nc) as tc:
        with (
            tc.tile_pool(name="constp", bufs=1) as constp,
            tc.tile_pool(name="iop", bufs=2) as iop,
            tc.tile_pool(name="psp", bufs=1, space="PSUM") as psp,
        ):
            SPR = constp.tile([128, NSPC, SPW], BF16)
            G3 = constp.tile([128, NUC, 3, KK], BF16)
            T6 = constp.tile([128, NUC, 2, 1024], BF16)
            SX6 = constp.tile([128, NGC, 2, KK], BF16)
            OUT = constp.tile([128, NGC], F32)
            VTA = constp.tile([128, NUC, 2, KK], BF16)
            VTB = constp.tile([128, NUC, 2, KK], BF16)

            # ---- consolidated input DMAs over 3 queues -----------------
            nc.sync.dma_start(SPR[:], spr[:])
            nc.scalar.dma_start(T6[:, 0], t6[:, 0])
            nc.gpsimd.dma_start(G3[:], g3[:])
            nc.scalar.dma_start(T6[:, 1], t6[:, 1])
            nc.gpsimd.dma_start(SX6[:], sx6[:])

            mm = nc.tensor.matmul

            # ---- P1 spread + P2 G-mult, per u-chunk --------------------
            ps_s = psp.tile([128, NUC, 2, KK], F32, tag="S")
            for ci in range(NUC):
                # one accumulation group open per psum bank at a time
                for ri, (b0, b1) in enumerate(((0, 2), (1, 0))):
                    for j in range(NSPC):
                        lh = lambda zri: SPR[:, j, zri * U4 + ci * 128:
                                             zri * U4 + (ci + 1) * 128]
                        rh = lambda b: SPR[:, j, 2 * U4 + b * KK:
                                           2 * U4 + (b + 1) * KK]
                        mm(ps_s[:, ci, ri], lh(0), rh(b0),
                           start=j == 0, stop=False)
                        mm(ps_s[:, ci, ri], lh(1), rh(b1),
                           start=False, stop=j == NSPC - 1)
                # VTA = [G*rho_r | -G*rho_i]   (one DVE op, psum -> bf16)
                nc.vector.tensor_tensor(VTA[:, ci], ps_s[:, ci],
                                        G3[:, ci, 0:2], op=mult)
                # VTB = [G*rho_i | G*rho_r]
                nc.vector.tensor_tensor(VTB[:, ci, 0:1], ps_s[:, ci, 1:2],
                                        G3[:, ci, 2:3], op=mult)
                nc.scalar.copy(VTB[:, ci, 1], VTA[:, ci, 0])

            # ---- P3 gather + P4 dot: two 4-bank waves, u-chunk outer ---
            for wv in range(2):
                wts = [psp.tile([128, 2, KK], F32, tag=f"W{gj}",
                                name=f"wt{wv}{gj}")
                       for gj in range(4)]
                for ci in range(NUC):
                    for gj in range(4):
                        gi = wv * 4 + gj
                        gsl = slice(gi * 128, (gi + 1) * 128)
                        mm(wts[gj][:], T6[:, ci, 0, gsl], VTA[:, ci],
                           start=ci == 0, stop=False)
                        mm(wts[gj][:], T6[:, ci, 1, gsl], VTB[:, ci],
                           start=False, stop=ci == NUC - 1)
                for gj in range(4):
                    gi = wv * 4 + gj
                    scr = iop.tile([128, 2, KK], BF16, tag=f"s{gj % 2}")
                    nc.vector.tensor_tensor(scr[:], wts[gj][:], SX6[:, gi],
                                            op=mult)
                    nc.vector.tensor_reduce(OUT[:, gi:gi + 1], scr[:],
                                            axis=mybir.AxisListType.XY, op=add)
                nc.sync.dma_start(outp[:, wv * 4:wv * 4 + 4],
                                  OUT[:, wv * 4:wv * 4 + 4])
    nc.compile()
    return nc


def _weights_order4(x):
    x2 = x * x
    x3 = x2 * x
    return np.stack([
        (1 - 6 * x + 12 * x2 - 8 * x3) / 48,
        (23 - 30 * x - 12 * x2 + 24 * x3) / 48,
        (23 + 30 * x - 12 * x2 - 24 * x3) / 48,
        (1 + 6 * x + 12 * x2 + 8 * x3) / 48,
    ])


def host_prep(cell, positions, charges):
    NA = positions.shape[0]
    NSP = charges.shape[1]
    cell = np.asarray(cell, dtype=np.float64)
    positions = np.asarray(positions, dtype=np.float64)
    charges = np.asarray(charges, dtype=np.float64)

    inv_cell = np.linalg.inv(cell)
    pos_rel = NS * (positions @ inv_cell)
    idx0 = np.floor(pos_rel)
    t = pos_rel - (idx0 + 0.5)
    w = _weights_order4(t)                                   # (4, NA, 3)
    offs = np.arange(-1, 3)
    idx = (idx0.astype(np.int64)[None] + offs[:, None, None]) % NS

    mo = np.arange(-K, K + 1)
    moz = np.arange(0, K + 1)

    def dft(ax, modes):
        ph = np.exp(-2j * np.pi * idx[:, :, ax][..., None] * modes / NS)
        return np.einsum("jn,jnm->nm", w[:, :, ax], ph)      # (NA, M)

    Sx = dft(0, mo)
    Sy = dft(1, mo)
    Sz = dft(2, moz)

    # smeared Coulomb kernel; Hermitian-z weight and 1/det folded in
    recip = 2 * np.pi * inv_cell.T
    kxg, kyg, kzg = np.meshgrid(mo.astype(np.float64), mo.astype(np.float64),
                                moz.astype(np.float64), indexing="ij")
    kvec = (kxg[..., None] * recip[0] + kyg[..., None] * recip[1]
            + kzg[..., None] * recip[2])
    ksq = np.sum(kvec * kvec, axis=-1)
    G = np.where(ksq == 0, 0.0,
                 4 * np.pi * np.exp(-0.5 * SMEARING ** 2 * ksq)
                 / np.where(ksq == 0, 1.0, ksq))
    G = G / np.abs(np.linalg.det(cell))
    wkz = np.where(moz == 0, 1.0, 2.0)
    Gw = G * wkz                                             # (KK, KK, KZ)

    # keep the UTOT lowest-|ky,kz| rows, dealt alternately to the halves
    r2 = (mo[:, None] ** 2 + moz[None, :] ** 2).ravel()
    order = np.argsort(r2, kind="stable")[:UTOT]
    halves = [order[0::2], order[1::2]]                      # U4 rows each
    yix = [h // KZ for h in halves]
    zix = [h % KZ for h in halves]

    sx6 = np.zeros((1024, 2, KK), dtype=NPBF)
    sx6[:NA, 0] = Sx.real
    sx6[:NA, 1] = -Sx.imag
    sx6 = np.ascontiguousarray(
        sx6.reshape(NGC, 128, 2, KK).transpose(1, 0, 2, 3))  # (128,NGC,2,KK)

    per_h = []
    for h in range(2):
        TYZ = Sy[:, yix[h]] * Sz[:, zix[h]]                  # (NA, U4)
        t6 = np.zeros((U4, 2, 1024), dtype=NPBF)
        t6[:, 0, :NA] = TYZ.real.T
        t6[:, 1, :NA] = TYZ.imag.T
        t6 = np.ascontiguousarray(
            t6.reshape(NUC, 128, 2, 1024).transpose(1, 0, 2, 3))
        gs = Gw[:, yix[h], zix[h]].T                         # (U4, KK)
        g3 = np.stack([gs, -gs, gs], axis=1).astype(NPBF)    # (U4, 3, KK)
        g3 = np.ascontiguousarray(
            g3.reshape(NUC, 128, 3, KK).transpose(1, 0, 2, 3))
        per_h.append((TYZ, t6, g3))

    NSPC = 1
    sels = []
    for c in range(NSP):
        sel = np.where(charges[:, c] != 0)[0]
        sels.append(sel)
        NSPC = max(NSPC, (len(sel) + 127) // 128)

    per_core = []
    for c in range(NSP):
        sel = sels[c]
        q = charges[sel, c]
        SxW = Sx[sel] * q[:, None]
        sxw = np.zeros((NSPC * 128, 3, KK))
        sxw[:len(sel), 0] = SxW.real
        sxw[:len(sel), 1] = SxW.imag
        sxw[:len(sel), 2] = -SxW.imag
        for h in range(2):
            TYZ = per_h[h][0]
            spra = np.zeros((NSPC * 128, SPW), dtype=NPBF)
            spra[:len(sel), 0:U4] = TYZ.real[sel]
            spra[:len(sel), U4:2 * U4] = TYZ.imag[sel]
            spra[:, 2 * U4:] = sxw.reshape(NSPC * 128, 3 * KK)
            spra = np.ascontiguousarray(
                spra.reshape(NSPC, 128, SPW).transpose(1, 0, 2))
            per_core.append({
                "spr": spra, "g3": per_h[h][2], "t6": per_h[h][1], "sx6": sx6,
            })
    return NSPC, NA, NSP, per_core


def _run(cell, positions, charges, trace=False):
    NSPC, NA, NSP, in_maps = host_prep(cell, positions, charges)
    if NSPC not in _cache:
        _cache[NSPC] = build_program(NSPC)
    nc = _cache[NSPC]

    res = run_bass_kernel_spmd(nc, in_maps, list(range(N_CORES)), trace=trace)

    pot = np.zeros((NA, NSP), dtype=np.float64)
    for core in range(N_CORES):
        c = core // 2
        out = res.results[core]["out"].astype(np.float64)    # (128, NGC)
        pot[:, c] += out.T.reshape(-1)[:NA]
    return pot.astype(np.float32), res


def kernel(cell, positions, charges):
    pot, _ = _run(cell, positions, charges, trace=False)
    return pot
